# revision 14
# baseline (speedup 1.0000x reference)
"""Haar DWT decoder (2-level inverse, zero details) as a Trainium2 Bass kernel.

out[b, c, j, k] = z[b].reshape(C, 128, 128)[c, j//4, k//4] * 0.25
i.e. a 4x4 nearest-neighbor upsample scaled by 1/4.

Data-parallel over batch: 128 samples -> 16 per core on 8 NeuronCores.
"""

import numpy as np

import concourse.bass as bass
import concourse.mybir as mybir
import concourse.tile as tile
from concourse.bass_utils import run_bass_kernel_spmd
# The walrus build in this container rejects instructions carrying more than
# one sync-wait command (codegen: "Too many sync wait commands" — observed on
# a Drain with 3 waits and a DMACopy with 2). Tile freely attaches several
# waits to one instruction, so after tracing we split the excess onto NOPs
# inserted just before the instruction on the same engine; sequential
# dispatch on one engine makes that equivalent.
_MAX_WAITS = 1


def _split_excess_waits(nc: bass.Bass) -> None:
    for f in nc.m.functions:
        for bb in f.blocks:
            insns = bb.instructions
            # Iterate over a snapshot; mutate the live list via insert.
            for ins in list(insns):
                si = ins.sync_info
                if si is None or not si.on_wait or len(si.on_wait) <= _MAX_WAITS:
                    continue
                waits = list(si.on_wait)
                keep = waits[-_MAX_WAITS:]
                spill = waits[:-_MAX_WAITS]
                pos = insns.index(ins)
                nops = []
                for i in range(0, len(spill), _MAX_WAITS):
                    nop = nc.engines[ins.engine].nop(nofuse=True).ins
                    # nop() appended itself to the current bb; pull it out.
                    cur = nc.cur_bb.bb.instructions
                    assert cur[-1] is nop
                    cur.pop()
                    nop.sync_info = mybir.SyncInfo(
                        on_wait=spill[i : i + _MAX_WAITS], on_update=[]
                    )
                    nops.append(nop)
                insns[pos:pos] = nops
                ins.sync_info = mybir.SyncInfo(
                    on_wait=keep, on_update=list(si.on_update)
                )

# Problem constants (hardcoded: module config out_shape=(3,512,512), levels=2)
BATCH = 128
C = 3
CAH = 128  # coarse-approximation spatial dims
CAW = 128
S = 4      # 2**levels upsample factor
H = 512
W = 512
N_CORES = 8
B_SHARD = BATCH // N_CORES  # 16

F32 = mybir.dt.float32


def _build_nc(b_shard: int = B_SHARD) -> bass.Bass:
    nc = bass.Bass("TRN2", target_bir_lowering=False, debug=False)
    z = nc.dram_tensor("z", [b_shard, C * CAH * CAW], F32, kind="ExternalInput").ap()
    out = nc.dram_tensor("out", [b_shard, C, H, W], F32, kind="ExternalOutput").ap()

    with tile.TileContext(nc) as tc:
        with (
            tc.tile_pool(name="zin", bufs=6) as zin_pool,
            tc.tile_pool(name="wide", bufs=6) as w_pool,
        ):
            dma_idx = 0
            for b in range(b_shard):
                # Load z[b] as [jc=128 partitions, (c, kc) free]. HWDGE only:
                # SWDGE (gpsimd) descriptor-ring traffic slows SDMA engines
                # 7/15 (shared AXI ports), leaving a straggler backlog.
                zt = zin_pool.tile([CAH, C * CAW], F32)
                src = z[b].rearrange("(c jc kc) -> jc c kc", c=C, jc=CAH, kc=CAW)
                ld_eng = nc.sync if b % 2 == 0 else nc.scalar
                ld_eng.dma_start(
                    out=zt[:].rearrange("p (c kc) -> p c kc", c=C), in_=src
                )
                zv = zt[:].rearrange("p (c kc) -> p c kc", c=C)

                # Materialize the upsampled sample in SBUF: partition jc holds
                # output rows 4*jc..4*jc+3 of every channel, free layout
                # (c, jr, k), so output DMAs are fully contiguous with 8 KiB
                # descriptor runs.
                w2 = w_pool.tile([CAH, C * S * W], F32, tag="wide")
                w2v = w2[:].rearrange(
                    "p (c jr kc kr) -> p c jr kc kr", c=C, jr=S, kc=CAW, kr=S
                )
                w2f = w2[:].rearrange("p (c jr k) -> p c jr k", c=C, jr=S)
                # Width-expand x4 (with the 1/4 scale) into the jr=0 rows in a
                # single contiguous-write op via a 0-stride (broadcast) input;
                # height-replicate into jr=1..3 split across DVE and ACT.
                zb = (
                    zv.unsqueeze(3).broadcast_to([CAH, C, CAW, S])
                )
                nc.vector.tensor_scalar_mul(w2v[:, :, 0, :, :], zb, 0.25)
                nc.scalar.copy(w2f[:, :, 1, :], w2f[:, :, 0, :])
                nc.vector.tensor_copy(w2f[:, :, 2, :], w2f[:, :, 0, :])
                nc.scalar.copy(w2f[:, :, 3, :], w2f[:, :, 0, :])

                # Output DMAs alternate between the two HWDGE rings. For the
                # first and last samples use per-channel 1 MiB DMAs (shorter
                # pipeline fill / drain); whole 3 MiB DMAs otherwise.
                if b == 0 or b == b_shard - 1:
                    for c in range(C):
                        ov = out[b, c].rearrange("(jc jr) k -> jc (jr k)", jr=S)
                        wc = w2[:].rearrange("p (c jrk) -> p c jrk", c=C)
                        eng = nc.sync if dma_idx % 2 == 0 else nc.scalar
                        dma_idx += 1
                        eng.dma_start(out=ov, in_=wc[:, c])
                else:
                    ov = out[b].rearrange("c (jc jr) k -> jc c (jr k)", jr=S)
                    eng = nc.sync if dma_idx % 2 == 0 else nc.scalar
                    dma_idx += 1
                    eng.dma_start(
                        out=ov, in_=w2[:].rearrange("p (c jrk) -> p c jrk", c=C)
                    )

    _split_excess_waits(nc)
    return nc


_NC_CACHE: dict[int, bass.Bass] = {}


def _get_nc(b_shard: int = B_SHARD) -> bass.Bass:
    if b_shard not in _NC_CACHE:
        _NC_CACHE[b_shard] = _build_nc(b_shard)
    return _NC_CACHE[b_shard]


def kernel(z: np.ndarray) -> np.ndarray:
    z = np.ascontiguousarray(z, dtype=np.float32)
    assert z.shape == (BATCH, C * CAH * CAW), z.shape
    nc = _get_nc()
    in_maps = [
        {"z": z[i * B_SHARD : (i + 1) * B_SHARD]} for i in range(N_CORES)
    ]
    res = run_bass_kernel_spmd(nc, in_maps, list(range(N_CORES)))
    return np.concatenate([res.results[i]["out"] for i in range(N_CORES)], axis=0)


# revision 16
# speedup vs baseline: 1.2421x; 1.2421x over previous
"""Haar DWT decoder (2-level inverse, zero details) as a Trainium2 Bass kernel.

out[b, c, j, k] = z[b].reshape(C, 128, 128)[c, j//4, k//4] * 0.25
i.e. a 4x4 nearest-neighbor upsample scaled by 1/4.

Data-parallel over batch: 128 samples -> 16 per core on 8 NeuronCores.
"""

import numpy as np

import concourse.bass as bass
import concourse.mybir as mybir
import concourse.tile as tile
from concourse.bass_utils import run_bass_kernel_spmd
# The walrus build in this container rejects instructions carrying more than
# one sync-wait command (codegen: "Too many sync wait commands" — observed on
# a Drain with 3 waits and a DMACopy with 2). Tile freely attaches several
# waits to one instruction, so after tracing we split the excess onto NOPs
# inserted just before the instruction on the same engine; sequential
# dispatch on one engine makes that equivalent.
_MAX_WAITS = 1


def _split_excess_waits(nc: bass.Bass) -> None:
    for f in nc.m.functions:
        for bb in f.blocks:
            insns = bb.instructions
            # Iterate over a snapshot; mutate the live list via insert.
            for ins in list(insns):
                si = ins.sync_info
                if si is None or not si.on_wait or len(si.on_wait) <= _MAX_WAITS:
                    continue
                waits = list(si.on_wait)
                keep = waits[-_MAX_WAITS:]
                spill = waits[:-_MAX_WAITS]
                pos = insns.index(ins)
                nops = []
                for i in range(0, len(spill), _MAX_WAITS):
                    nop = nc.engines[ins.engine].nop(nofuse=True).ins
                    # nop() appended itself to the current bb; pull it out.
                    cur = nc.cur_bb.bb.instructions
                    assert cur[-1] is nop
                    cur.pop()
                    nop.sync_info = mybir.SyncInfo(
                        on_wait=spill[i : i + _MAX_WAITS], on_update=[]
                    )
                    nops.append(nop)
                insns[pos:pos] = nops
                ins.sync_info = mybir.SyncInfo(
                    on_wait=keep, on_update=list(si.on_update)
                )

# Problem constants (hardcoded: module config out_shape=(3,512,512), levels=2)
BATCH = 128
C = 3
CAH = 128  # coarse-approximation spatial dims
CAW = 128
S = 4      # 2**levels upsample factor
H = 512
W = 512
N_CORES = 8
B_SHARD = BATCH // N_CORES  # 16

F32 = mybir.dt.float32


def _build_nc(b_shard: int = B_SHARD) -> bass.Bass:
    nc = bass.Bass("TRN2", target_bir_lowering=False, debug=False)
    z = nc.dram_tensor("z", [b_shard, C * CAH * CAW], F32, kind="ExternalInput").ap()
    out = nc.dram_tensor("out", [b_shard, C, H, W], F32, kind="ExternalOutput").ap()

    with tile.TileContext(nc) as tc:
        with (
            tc.tile_pool(name="zin", bufs=6) as zin_pool,
            tc.tile_pool(name="wide", bufs=6) as w_pool,
        ):
            dma_idx = 0
            for b in range(b_shard):
                # Load z[b] as [jc=128 partitions, (c, kc) free] via SWDGE
                # (gpsimd): the HWDGE rings execute FIFO per ring, so loads
                # there would queue behind multi-MiB output DMAs and stall the
                # pipeline. Keep zin bufs modest — front-loading all SWDGE
                # loads floods the descriptor rings whose SBUF AXI ports are
                # shared with SDMA engines 7/15, creating a straggler backlog.
                zt = zin_pool.tile([CAH, C * CAW], F32)
                src = z[b].rearrange("(c jc kc) -> jc c kc", c=C, jc=CAH, kc=CAW)
                nc.gpsimd.dma_start(
                    out=zt[:].rearrange("p (c kc) -> p c kc", c=C), in_=src
                )
                zv = zt[:].rearrange("p (c kc) -> p c kc", c=C)

                # Materialize the upsampled sample in SBUF: partition jc holds
                # output rows 4*jc..4*jc+3 of every channel, free layout
                # (c, jr, k), so output DMAs are fully contiguous with 8 KiB
                # descriptor runs.
                w2 = w_pool.tile([CAH, C * S * W], F32, tag="wide")
                w2v = w2[:].rearrange(
                    "p (c jr kc kr) -> p c jr kc kr", c=C, jr=S, kc=CAW, kr=S
                )
                w2f = w2[:].rearrange("p (c jr k) -> p c jr k", c=C, jr=S)
                # Width-expand x4 (with the 1/4 scale) into the jr=0 rows in a
                # single contiguous-write op via a 0-stride (broadcast) input;
                # height-replicate into jr=1..3 split across DVE and ACT.
                zb = (
                    zv.unsqueeze(3).broadcast_to([CAH, C, CAW, S])
                )
                nc.vector.tensor_scalar_mul(w2v[:, :, 0, :, :], zb, 0.25)
                nc.scalar.copy(w2f[:, :, 1, :], w2f[:, :, 0, :])
                nc.vector.tensor_copy(w2f[:, :, 2, :], w2f[:, :, 0, :])
                nc.scalar.copy(w2f[:, :, 3, :], w2f[:, :, 0, :])

                # One fully-contiguous 3 MiB DMA per sample; alternate between
                # the two HWDGE rings for descriptor-gen overlap.
                ov = out[b].rearrange("c (jc jr) k -> jc c (jr k)", jr=S)
                eng = nc.sync if dma_idx % 2 == 0 else nc.scalar
                dma_idx += 1
                eng.dma_start(
                    out=ov, in_=w2[:].rearrange("p (c jrk) -> p c jrk", c=C)
                )

    _split_excess_waits(nc)
    return nc


_NC_CACHE: dict[int, bass.Bass] = {}


def _get_nc(b_shard: int = B_SHARD) -> bass.Bass:
    if b_shard not in _NC_CACHE:
        _NC_CACHE[b_shard] = _build_nc(b_shard)
    return _NC_CACHE[b_shard]


def kernel(z: np.ndarray) -> np.ndarray:
    z = np.ascontiguousarray(z, dtype=np.float32)
    assert z.shape == (BATCH, C * CAH * CAW), z.shape
    nc = _get_nc()
    in_maps = [
        {"z": z[i * B_SHARD : (i + 1) * B_SHARD]} for i in range(N_CORES)
    ]
    res = run_bass_kernel_spmd(nc, in_maps, list(range(N_CORES)))
    return np.concatenate([res.results[i]["out"] for i in range(N_CORES)], axis=0)
